# revision 1
# baseline (speedup 1.0000x reference)
"""Trainium2 Bass kernel for nn_EquivariantModel (e3nn-style equivariant net).

Architecture (per batch row): two blocks of
  {o3.Linear x2 -> FullyConnectedTensorProduct('Mx0e+Mx1o' ^2 -> 128x0e+128x1o)
   -> learnable tanh gate -> o3.Linear}, then a final o3.Linear.

Strategy: data-parallel over batch (8 cores x 1024 rows), feature-major
activations [feature, batch] on-device.  The tensor product is computed as
z[(u,v), b] = f1[u,b] * f2[v,b] (f16, formed on DVE with PE-assisted
partition broadcast of the f1 factor), followed by PSUM-accumulated
matmuls z^T @ W[(u,v), w] over k-tiles.  Linears/gates run in fp32.
All normalization constants are folded into the weights host-side.
"""

import sys
import numpy as np

if '/opt/trn_rl_repo' not in sys.path:
    sys.path.insert(0, '/opt/trn_rl_repo')

B, M_IN, M_HID = 8192, 64, 128
N_CORES = 8
BC = B // N_CORES            # batch per core
CH = 512                     # chunk of batch processed per matmul group
NCH = BC // CH
TANH_GAIN = 1.5927116870880127

F32 = None  # set after mybir import
BF16 = None

_CACHE = {}


def _build_program(repeat=1):
    import concourse.mybir as mybir
    import concourse.tile as tile
    from concourse import bacc
    from contextlib import ExitStack

    f32 = mybir.dt.float32
    f16 = mybir.dt.float16

    nc = bacc.Bacc("TRN2", target_bir_lowering=False)

    # ---- DRAM I/O ----
    s0 = nc.dram_tensor("s0", [64, BC], f32, kind="ExternalInput")
    v0 = nc.dram_tensor("v0", [192, BC], f32, kind="ExternalInput")  # rows i*64+u

    dram = {}
    for blk, M in (("b1", 64), ("b2", 128)):
        for nm in ("l1_w0", "l1_w1", "l2_w0", "l2_w1"):
            dram[f"{blk}_{nm}"] = nc.dram_tensor(f"{blk}_{nm}", [M, M], f32,
                                                 kind="ExternalInput")
        K = M * M
        for nm in ("ss", "vv", "sv", "vs"):
            # p-major layout: [128, K//128, 128]
            dram[f"{blk}_w_{nm}"] = nc.dram_tensor(
                f"{blk}_w_{nm}", [128, K // 128, 128], f16, kind="ExternalInput")
        for nm in ("g_ws", "g_wg", "g_wv", "o_w0", "o_w1"):
            dram[f"{blk}_{nm}"] = nc.dram_tensor(f"{blk}_{nm}", [128, 128], f32,
                                                 kind="ExternalInput")
    dram["f_w0"] = nc.dram_tensor("f_w0", [128, 64], f32, kind="ExternalInput")
    dram["f_w1"] = nc.dram_tensor("f_w1", [128, 64], f32, kind="ExternalInput")
    idm = nc.dram_tensor("idm", [128, 128], f16, kind="ExternalInput")
    di64 = nc.dram_tensor("di64", [64, 4096], f16, kind="ExternalInput")

    out_d = nc.dram_tensor("out", [256, BC], f32, kind="ExternalOutput")

    with ExitStack() as ctx:
        tc = ctx.enter_context(tile.TileContext(nc))
        consts = ctx.enter_context(tc.tile_pool(name="consts", bufs=1))
        acts = ctx.enter_context(tc.tile_pool(name="acts", bufs=1))
        wstream = ctx.enter_context(tc.tile_pool(name="wstream", bufs=2))
        bc_ps = ctx.enter_context(tc.tile_pool(name="bc_ps", bufs=4, space="PSUM"))
        acc_ps = ctx.enter_context(tc.tile_pool(name="acc_ps", bufs=1, space="PSUM"))
        bc_sb = ctx.enter_context(tc.tile_pool(name="bc_sb", bufs=2))
        z_pool = ctx.enter_context(tc.tile_pool(name="z", bufs=6))
        tmp = ctx.enter_context(tc.tile_pool(name="tmp", bufs=3))

        # ---- load constants ----
        W = {}
        for name, t in dram.items():
            if name.startswith("b1_w_"):
                w = consts.tile([128, 32, 128], f16, tag=name, name=name)
                nc.sync.dma_start(w[:], t[:])
                W[name] = w
            elif name.startswith("b2_w_"):
                W[name] = t  # streamed
            else:
                shp = list(t.shape)
                w = consts.tile(shp, f32, tag=name, name=name)
                nc.sync.dma_start(w[:], t[:])
                W[name] = w
        id_sb = consts.tile([128, 128], f16, tag="idm", name="idm")
        nc.sync.dma_start(id_sb[:], idm[:])
        di_sb = consts.tile([64, 4096], f16, tag="di64", name="di64")
        nc.sync.dma_start(di_sb[:], di64[:])

        # ---- input activations ----
        sT = acts.tile([64, BC], f32, tag="in_s", name="in_s")
        nc.sync.dma_start(sT[:], s0[:])
        vT = []
        for i in range(3):
            t = acts.tile([64, BC], f32, tag=f"in_v{i}", name=f"in_v{i}")
            nc.sync.dma_start(t[:], v0[i * 64:(i + 1) * 64, :])
            vT.append(t)

        def linear(w_sb, x_sb, Min, Mout, out_sb, out_rows=None, second_rows=None):
            """out = w^T x, feature-major; optional duplicate write to rows."""
            for c in range(NCH):
                sl = slice(c * CH, (c + 1) * CH)
                ps = bc_ps.tile([128, CH], f32, tag="bc", name="bc")
                nc.tensor.matmul(ps[:Mout], w_sb[:Min, :Mout], x_sb[:Min, sl],
                                 start=True, stop=True)
                r0 = out_rows or slice(0, Mout)
                nc.scalar.copy(out_sb[r0, sl], ps[:Mout])
                if second_rows is not None:
                    nc.scalar.copy(out_sb[second_rows, sl], ps[:Mout])

        def block(blk, U, s_in, v_in, s_out, v_out):
            """One equivariant block. s_in [U, BC] f32, v_in [3][U, BC] f32.
            Writes s_out [128, BC] f32, v_out [3][128, BC] f32."""
            V = U
            K = U * V
            KT = K // 128
            g = 128 // V

            # --- l1 / l2 linears -> bf16 operands ---
            # bcast-side factors (f1): s1b [U, BC], v1b[i] [U, BC]
            s1b = acts.tile([U, BC], f16, tag="s1b", name="s1b")
            v1b = [acts.tile([U, BC], f16, tag=f"v1b{i}", name=f"v1b{i}") for i in range(3)]
            # tile-side factors (f2), partition-replicated to 128 rows
            s2r = acts.tile([128, BC], f16, tag="s2r", name="s2r")
            v2r = [acts.tile([128, BC], f16, tag=f"v2r{i}", name=f"v2r{i}") for i in range(3)]

            dup = slice(64, 128) if g == 2 else None
            linear(W[f"{blk}_l1_w0"], s_in, U, U, s1b)
            for i in range(3):
                linear(W[f"{blk}_l1_w1"], v_in[i], U, U, v1b[i])
            linear(W[f"{blk}_l2_w0"], s_in, U, U, s2r, second_rows=dup)
            for i in range(3):
                linear(W[f"{blk}_l2_w1"], v_in[i], U, U, v2r[i], second_rows=dup)

            # --- tensor product ---
            tp_s = acts.tile([128, BC], f32, tag="tp_s", name="tp_s")
            tp_v = [acts.tile([128, BC], f32, tag=f"tp_v{i}", name=f"tp_v{i}") for i in range(3)]

            for c in range(NCH):
                sl = slice(c * CH, (c + 1) * CH)
                acc_s = acc_ps.tile([128, CH], f32, tag="acc_s", name="acc_s")
                acc_v = [acc_ps.tile([128, CH], f32, tag=f"acc_v{i}", name=f"acc_v{i}")
                         for i in range(3)]
                for kt in range(KT):
                    u0 = kt * g
                    if blk == "b1":
                        wss = W["b1_w_ss"][:, kt]
                        wvv = W["b1_w_vv"][:, kt]
                        wsv = W["b1_w_sv"][:, kt]
                        wvs = W["b1_w_vs"][:, kt]
                    else:
                        wss = wstream.tile([128, 128], f16, tag="wss", name="wss")
                        nc.sync.dma_start(wss[:], W["b2_w_ss"][:, kt])
                        wvv = wstream.tile([128, 128], f16, tag="wvv", name="wvv")
                        nc.sync.dma_start(wvv[:], W["b2_w_vv"][:, kt])
                        wsv = wstream.tile([128, 128], f16, tag="wsv", name="wsv")
                        nc.sync.dma_start(wsv[:], W["b2_w_sv"][:, kt])
                        wvs = wstream.tile([128, 128], f16, tag="wvs", name="wvs")
                        nc.sync.dma_start(wvs[:], W["b2_w_vs"][:, kt])

                    # partition-broadcast of f1 rows via selector matmul
                    if g == 2:
                        sel = di_sb[:64, 64 * u0: 64 * u0 + 128]
                    else:
                        sel = id_sb[:, u0:u0 + 1].to_broadcast((128, 128))
                    bps = bc_ps.tile([128, CH], f32, tag="bc", name="bc")
                    nc.tensor.matmul(bps, sel, s1b[:U, sl],
                                     start=True, stop=True)
                    bs = bc_sb.tile([128, CH], f16, tag="bcs", name="bcs")
                    nc.scalar.copy(bs, bps)
                    bv = []
                    for i in range(3):
                        p = bc_ps.tile([128, CH], f32, tag="bc", name="bc")
                        nc.tensor.matmul(p, sel, v1b[i][:U, sl],
                                         start=True, stop=True)
                        t = bc_sb.tile([128, CH], f16, tag=f"bcv{i}", name=f"bcv{i}")
                        nc.scalar.copy(t, p)
                        bv.append(t)

                    first = kt == 0
                    last = kt == KT - 1
                    # scalar output: ss + vv paths accumulate into acc_s
                    z = z_pool.tile([128, CH], f16, tag="z", name="z")
                    nc.vector.tensor_mul(z, bs, s2r[:, sl])
                    nc.tensor.matmul(acc_s, wss, z, start=first, stop=False)
                    for i in range(3):
                        z = z_pool.tile([128, CH], f16, tag="z", name="z")
                        nc.vector.tensor_mul(z, bv[i], v2r[i][:, sl])
                        nc.tensor.matmul(acc_s, wvv, z, start=False,
                                         stop=(last and i == 2))
                    # vector outputs: sv + vs paths
                    for i in range(3):
                        z = z_pool.tile([128, CH], f16, tag="z", name="z")
                        nc.vector.tensor_mul(z, bs, v2r[i][:, sl])
                        nc.tensor.matmul(acc_v[i], wsv, z, start=first, stop=False)
                    for i in range(3):
                        z = z_pool.tile([128, CH], f16, tag="z", name="z")
                        nc.vector.tensor_mul(z, bv[i], s2r[:, sl])
                        nc.tensor.matmul(acc_v[i], wvs, z, start=False, stop=last)

                nc.vector.tensor_copy(tp_s[:, sl], acc_s)
                for i in range(3):
                    nc.vector.tensor_copy(tp_v[i][:, sl], acc_v[i])

            # --- gate ---
            tanh_s = acts.tile([128, BC], f32, tag="tanh_s", name="tanh_s")
            gated_v = [acts.tile([128, BC], f32, tag=f"gated_v{i}", name=f"gated_v{i}")
                       for i in range(3)]
            for c in range(NCH):
                sl = slice(c * CH, (c + 1) * CH)
                ps = bc_ps.tile([128, CH], f32, tag="bc", name="bc")
                nc.tensor.matmul(ps, W[f"{blk}_g_ws"], tp_s[:, sl],
                                 start=True, stop=True)
                nc.scalar.activation(tanh_s[:, sl], ps,
                                     mybir.ActivationFunctionType.Tanh)
                psg = bc_ps.tile([128, CH], f32, tag="bc", name="bc")
                nc.tensor.matmul(psg, W[f"{blk}_g_wg"], tp_s[:, sl],
                                 start=True, stop=True)
                tg = tmp.tile([128, CH], f32, tag="tanh_g", name="tanh_g")
                nc.scalar.activation(tg, psg,
                                     mybir.ActivationFunctionType.Tanh)
                for i in range(3):
                    psv = bc_ps.tile([128, CH], f32, tag="bc", name="bc")
                    nc.tensor.matmul(psv, W[f"{blk}_g_wv"], tp_v[i][:, sl],
                                     start=True, stop=True)
                    nc.vector.tensor_mul(gated_v[i][:, sl], psv, tg)

            # --- out linear ---
            linear(W[f"{blk}_o_w0"], tanh_s, 128, 128, s_out)
            for i in range(3):
                linear(W[f"{blk}_o_w1"], gated_v[i], 128, 128, v_out[i])

        def _network():
            # block 1, block 2, final linear
            s_b1 = acts.tile([128, BC], f32, tag="s_mid", name="s_mid")
            v_b1 = [acts.tile([128, BC], f32, tag=f"v_mid{i}", name=f"v_mid{i}") for i in range(3)]
            block("b1", 64, sT, vT, s_b1, v_b1)
            s_b2 = acts.tile([128, BC], f32, tag="s_mid2", name="s_mid2")
            v_b2 = [acts.tile([128, BC], f32, tag=f"v_mid2{i}", name=f"v_mid2{i}") for i in range(3)]
            block("b2", 128, s_b1, v_b1, s_b2, v_b2)
            fo_a = acts.tile([128, BC], f32, tag="final_a", name="final_a")
            fo_b = acts.tile([128, BC], f32, tag="final_b", name="final_b")
            linear(W["f_w0"], s_b2, 128, 64, fo_a, out_rows=slice(0, 64))
            linear(W["f_w1"], v_b2[0], 128, 64, fo_a, out_rows=slice(64, 128))
            linear(W["f_w1"], v_b2[1], 128, 64, fo_b, out_rows=slice(0, 64))
            linear(W["f_w1"], v_b2[2], 128, 64, fo_b, out_rows=slice(64, 128))
            nc.sync.dma_start(out_d[0:128, :], fo_a[:])
            nc.sync.dma_start(out_d[128:256, :], fo_b[:])

        # repeat>1 wraps the network in an on-device loop (used only by
        # test.py for precise per-iteration timing; the grading path uses 1)
        if repeat > 1:
            with tc.For_i(0, repeat, 1):
                _network()
        else:
            _network()

    nc.finalize()
    return nc


def _host_prep(inputs):
    """Fold norm constants into weights; reorder/cast TP weights."""
    hf = np.float16
    d = {}
    for blk, M in (("b1", 64), ("b2", 128)):
        c_lin = np.float32(1.0 / np.sqrt(M))
        for nm in ("l1_w0", "l1_w1", "l2_w0", "l2_w1"):
            d[f"{blk}_{nm}"] = np.ascontiguousarray(
                inputs[f"{blk}_{nm}"] * c_lin, dtype=np.float32)
        c_tp = 1.0 / (M * np.sqrt(2.0))
        for nm, c in (("ss", c_tp), ("vv", c_tp / np.sqrt(3.0)),
                      ("sv", c_tp), ("vs", c_tp)):
            W = (inputs[f"{blk}_tp_{nm}"] * np.float32(c)).reshape(M * M, 128)
            # p-major: [128, K//128, 128]
            Wp = W.reshape(M * M // 128, 128, 128).transpose(1, 0, 2)
            d[f"{blk}_w_{nm}"] = np.ascontiguousarray(Wp).astype(hf)
        c_g = np.float32(1.0 / np.sqrt(128))
        for nm in ("g_ws", "g_wg", "g_wv"):
            d[f"{blk}_{nm}"] = np.ascontiguousarray(
                inputs[f"{blk}_{nm}"] * c_g, dtype=np.float32)
        c_og = np.float32(TANH_GAIN / np.sqrt(128))
        d[f"{blk}_o_w0"] = np.ascontiguousarray(
            inputs[f"{blk}_o_w0"] * c_og, dtype=np.float32)
        d[f"{blk}_o_w1"] = np.ascontiguousarray(
            inputs[f"{blk}_o_w1"] * c_og, dtype=np.float32)
    c_o = np.float32(1.0 / np.sqrt(128))
    d["f_w0"] = np.ascontiguousarray(inputs["f_w0"] * c_o, dtype=np.float32)
    d["f_w1"] = np.ascontiguousarray(inputs["f_w1"] * c_o, dtype=np.float32)
    d["idm"] = np.eye(128, dtype=np.float32).astype(hf)
    di = np.zeros((64, 4096), dtype=np.float32)
    for u in range(64):
        di[u, 64 * u:64 * u + 64] = 1.0
    d["di64"] = di.astype(hf)
    return d


def kernel(**inputs):
    from concourse.bass_utils import run_bass_kernel_spmd

    x = np.asarray(inputs["x"], dtype=np.float32)
    w = _host_prep({k: np.asarray(v, dtype=np.float32)
                    for k, v in inputs.items() if k != "x"})

    if "nc" not in _CACHE:
        _CACHE["nc"] = _build_program()
    nc = _CACHE["nc"]

    # shard + transpose to feature-major
    sT_full = np.ascontiguousarray(x[:, :64].T)                  # [64, B]
    v_full = x[:, 64:].reshape(B, 64, 3)
    vT_full = np.ascontiguousarray(v_full.transpose(2, 1, 0))    # [3, 64, B]
    in_maps = []
    for c in range(N_CORES):
        bs = slice(c * BC, (c + 1) * BC)
        m = dict(w)
        m["s0"] = np.ascontiguousarray(sT_full[:, bs])
        m["v0"] = np.ascontiguousarray(vT_full[:, :, bs]).reshape(192, BC)
        in_maps.append(m)

    res = run_bass_kernel_spmd(nc, in_maps, core_ids=list(range(N_CORES)))

    out = np.empty((B, 256), dtype=np.float32)
    for c in range(N_CORES):
        o = res.results[c]["out"]                                # [256, BC]
        bs = slice(c * BC, (c + 1) * BC)
        out[bs, :64] = o[:64].T
        # rows 64+64i+u = v comp i; ref layout col 64 + u*3 + i
        v = o[64:].reshape(3, 64, BC)
        out[bs, 64:] = v.transpose(2, 1, 0).reshape(BC, 192)
    return out



# revision 2
# speedup vs baseline: 1.0893x; 1.0893x over previous
"""Trainium2 Bass kernel v2 for nn_EquivariantModel (e3nn-style equivariant net).

Data-parallel over batch (8 cores x 1024 rows), feature-major activations
[feature, batch].  Per-block tensor product z[(u,v),b] = f1[u,b]*f2[v,b]
is formed with:
  - partition-broadcast of f1 rows via DMA from a DRAM staging copy
    (frees PE + scalar engine from broadcast work),
  - grouped DVE f16 multiplies (3 fused 3D-AP tensor_tensor ops per kt,
    2x perf mode), with a tunable slice offloaded to GPSIMD,
  - PSUM-accumulated matmuls over k-tiles.
Out-linears are folded into the next block's l1/l2 linears host-side.
"""

import sys
import numpy as np

if '/opt/trn_rl_repo' not in sys.path:
    sys.path.insert(0, '/opt/trn_rl_repo')

B, M_IN, M_HID = 8192, 64, 128
N_CORES = 8
BC = B // N_CORES            # batch per core
CH = 512                     # chunk of batch per matmul / mul group
NCH = BC // CH
TANH_GAIN = 1.5927116870880127
G_W = 8                      # b2 weight kts per stream DMA
LDW_SKIP = True              # skip redundant weight loads within a weight run

_CACHE = {}


def _build_program(repeat=1):
    import concourse.mybir as mybir
    import concourse.tile as tile
    from concourse import bacc
    from contextlib import ExitStack

    f32 = mybir.dt.float32
    f16 = mybir.dt.float16
    Tanh = mybir.ActivationFunctionType.Tanh

    nc = bacc.Bacc("TRN2", target_bir_lowering=False)

    # ---- DRAM I/O ----
    s0 = nc.dram_tensor("s0", [64, BC], f16, kind="ExternalInput")
    v0 = nc.dram_tensor("v0", [192, BC], f16, kind="ExternalInput")

    # TP weights, k-major concatenated: [128, KT*4*128] f16
    wtp1_d = nc.dram_tensor("wtp1", [128, 32 * 4 * 128], f16, kind="ExternalInput")
    wtp2_d = nc.dram_tensor("wtp2", [128, 128 * 4 * 128], f16, kind="ExternalInput")

    # small linear / gate weights
    lw = {}
    for nm, shp in (
        ("l1_s", (64, 64)), ("l1_v", (64, 64)), ("l2_s", (64, 64)), ("l2_v", (64, 64)),
        ("g1_ws", (128, 128)), ("g1_wg", (128, 128)), ("g1_wv", (128, 128)),
        ("f2_l1_s", (128, 128)), ("f2_l1_v", (128, 128)),
        ("f2_l2_s", (128, 128)), ("f2_l2_v", (128, 128)),
        ("g2_ws", (128, 128)), ("g2_wg", (128, 128)), ("g2_wv", (128, 128)),
        ("ff_s", (128, 64)), ("ff_v", (128, 64)),
    ):
        lw[nm] = nc.dram_tensor(nm, list(shp), f16, kind="ExternalInput")

    # DRAM staging for broadcast factors [U, 4*BC]
    f1d = {
        "b1": nc.dram_tensor("f1d_b1", [64, 4 * BC], f16, kind="Internal"),
        "b2": nc.dram_tensor("f1d_b2", [128, 4 * BC], f16, kind="Internal"),
    }

    out_d = nc.dram_tensor("out", [256, BC], f32, kind="ExternalOutput")

    with ExitStack() as ctx:
        tc = ctx.enter_context(tile.TileContext(nc))
        consts = ctx.enter_context(tc.tile_pool(name="consts", bufs=1))
        acts = ctx.enter_context(tc.tile_pool(name="acts", bufs=1))
        wstream = ctx.enter_context(tc.tile_pool(name="wstream", bufs=2))
        bc_pool = ctx.enter_context(tc.tile_pool(name="bcast", bufs=3))
        zA_pool = ctx.enter_context(tc.tile_pool(name="zA", bufs=2))
        zB_pool = ctx.enter_context(tc.tile_pool(name="zB", bufs=2))
        zC_pool = ctx.enter_context(tc.tile_pool(name="zC", bufs=2))
        # single PSUM pool: 8 banks as 8 rotating [128, 512] tags
        psum = ctx.enter_context(tc.tile_pool(name="psum", bufs=1, space="PSUM"))

        def pstile(tag):
            return psum.tile([128, CH], f32, tag=tag, name=tag)

        # ---- constants ----
        W = {}
        for nm, t in lw.items():
            w = consts.tile(list(t.shape), t.dtype, tag=nm, name=nm)
            nc.sync.dma_start(w[:], t[:])
            W[nm] = w
        wb1 = consts.tile([128, 32 * 4 * 128], f16, tag="wb1", name="wb1")
        nc.sync.dma_start(wb1[:], wtp1_d[:])

        # ---- input activations ----
        sT = acts.tile([64, BC], f16, tag="in_s", name="in_s")
        nc.sync.dma_start(sT[:], s0[:])
        vT = []
        for i in range(3):
            t = acts.tile([64, BC], f16, tag=f"in_v{i}", name=f"in_v{i}")
            nc.sync.dma_start(t[:], v0[i * 64:(i + 1) * 64, :])
            vT.append(t)

        def tp_block(blk, U, KT, f1_all, f2_all, wtile_fn):
            """TP loop (kt outer, full BC per kt): returns tp_s, tp_v f16."""
            tp_s = acts.tile([128, BC], f16, tag="tp_s", name="tp_s")
            tp_v = acts.tile([128, 3 * BC], f16, tag="tp_v", name="tp_v")
            tp_vv = tp_v[:].rearrange("p (s f) -> p s f", s=3)

            f2v = f2_all[:].rearrange("p (s f) -> p s f", s=4)  # [128,4,BC]

            # accs: [quantity][chunk] -> [128, CH] psum tile
            acc = {(q, c): pstile(f"acc{q}{c}")
                   for q in range(4) for c in range(NCH)}
            chunks = [(c, slice(c * CH, (c + 1) * CH)) for c in range(NCH)]
            for kt in range(KT):
                # broadcast tile [128, 4, BC] from DRAM staging
                bcast = bc_pool.tile([128, 4 * BC], f16, tag="bc", name="bc")
                bcv = bcast[:].rearrange("p (s f) -> p s f", s=4)
                if U == 128:
                    nc.sync.dma_start(
                        bcast[:], f1d[blk][kt:kt + 1, :].partition_broadcast(128))
                else:
                    nc.sync.dma_start(
                        bcast[0:64, :],
                        f1d[blk][2 * kt:2 * kt + 1, :].partition_broadcast(64))
                    nc.sync.dma_start(
                        bcast[64:128, :],
                        f1d[blk][2 * kt + 1:2 * kt + 2, :].partition_broadcast(64))

                # group A: {ss, sv0, sv1, sv2} = bc_s * (s2, v2_i)
                zA = zA_pool.tile([128, 4 * BC], f16, tag="zA", name="zA")
                zAv = zA[:].rearrange("p (s f) -> p s f", s=4)
                nc.vector.tensor_mul(
                    zAv, bcv[:, 0:1, :].to_broadcast((128, 4, BC)), f2v)
                # group C: {vv0..2} = bc_v_i * v2_i
                zC = zC_pool.tile([128, 3 * BC], f16, tag="zC", name="zC")
                zCv = zC[:].rearrange("p (s f) -> p s f", s=3)
                nc.vector.tensor_mul(zCv, bcv[:, 1:4, :], f2v[:, 1:4, :])
                # group B: {vs0..2} = bc_v_i * s2
                zB = zB_pool.tile([128, 3 * BC], f16, tag="zB", name="zB")
                zBv = zB[:].rearrange("p (s f) -> p s f", s=3)
                nc.vector.tensor_mul(
                    zBv, bcv[:, 1:4, :],
                    f2v[:, 0:1, :].to_broadcast((128, 3, BC)))

                wss, wvv, wsv, wvs = wtile_fn(kt)
                first, last = kt == 0, kt == KT - 1
                # weight-outer runs; skip redundant weight loads within a run
                runs = (
                    [(wss, [(acc[0, c], zAv[:, 0, sl], first, False)
                            for c, sl in chunks])] +
                    [(wvv, [(acc[0, c], zCv[:, i, sl], False, last and i == 2)
                            for i in range(3) for c, sl in chunks])] +
                    [(wsv, [(acc[1 + i, c], zAv[:, 1 + i, sl], first, False)
                            for i in range(3) for c, sl in chunks])] +
                    [(wvs, [(acc[1 + i, c], zBv[:, i, sl], False, last)
                            for i in range(3) for c, sl in chunks])]
                )
                for w, mms in runs:
                    for j, (dst, src, st, sp) in enumerate(mms):
                        inst = nc.tensor.matmul(dst, w, src, start=st, stop=sp)
                        if LDW_SKIP and j > 0:
                            inst.ins.ldweights = False

            for c in range(NCH):
                sl = slice(c * CH, (c + 1) * CH)
                nc.scalar.copy(tp_s[:, sl], acc[0, c][:])
                for i in range(3):
                    nc.scalar.copy(tp_vv[:, i, sl], acc[1 + i, c][:])
            return tp_s, tp_v, tp_vv

        def gate(gws, gwg, gwv, tp_s, tp_vv):
            """-> tanh_s [128,BC] f16, gated [128,3*BC] f16 (grouped)."""
            tanh_s = acts.tile([128, BC], f16, tag="tanh_s", name="tanh_s")
            tg = acts.tile([128, BC], f16, tag="tg", name="tg")
            gv = acts.tile([128, 3 * BC], f16, tag="gv", name="gv")
            gvv = gv[:].rearrange("p (s f) -> p s f", s=3)
            gated = acts.tile([128, 3 * BC], f16, tag="gated", name="gated")
            gatedv = gated[:].rearrange("p (s f) -> p s f", s=3)
            chunks = [(c, slice(c * CH, (c + 1) * CH)) for c in range(NCH)]
            for c, sl in chunks:
                p1 = pstile(f"acc0{c}")
                inst = nc.tensor.matmul(p1, gws, tp_s[:, sl],
                                        start=True, stop=True)
                if LDW_SKIP and c > 0:
                    inst.ins.ldweights = False
                nc.scalar.activation(tanh_s[:, sl], p1[:], Tanh)
            for c, sl in chunks:
                p2 = pstile(f"acc1{c}")
                inst = nc.tensor.matmul(p2, gwg, tp_s[:, sl],
                                        start=True, stop=True)
                if LDW_SKIP and c > 0:
                    inst.ins.ldweights = False
                nc.scalar.activation(tg[:, sl], p2[:], Tanh)
            k = 0
            for i in range(3):
                for c, sl in chunks:
                    p3 = pstile(f"acc{2 + k % 2}{(k // 2) % 2}")
                    inst = nc.tensor.matmul(p3, gwv, tp_vv[:, i, sl],
                                            start=True, stop=True)
                    if LDW_SKIP and k > 0:
                        inst.ins.ldweights = False
                    nc.scalar.copy(gvv[:, i, sl], p3[:])
                    k += 1
            tgv = tg[:].rearrange("p (s f) -> p s f", s=1)
            nc.vector.tensor_mul(gatedv, gvv,
                                 tgv.to_broadcast((128, 3, BC)))
            return tanh_s, gated, gatedv

        def factors(prefix, s_src, v_slices, Min, U, blk):
            """fused l1/l2 linears -> f1_all/f2_all + stage f1 to DRAM."""
            f1_all = acts.tile([128, 4 * BC], f16, tag="f1_all", name="f1_all")
            f2_all = acts.tile([128, 4 * BC], f16, tag="f2_all", name="f2_all")
            f1v = f1_all[:].rearrange("p (s f) -> p s f", s=4)
            f2v = f2_all[:].rearrange("p (s f) -> p s f", s=4)
            w1s, w1v, w2s, w2v = prefix
            dup = U == 64
            srcs = [s_src] + list(v_slices)
            chunks = [(c, slice(c * CH, (c + 1) * CH)) for c in range(NCH)]
            k = 0
            for j in range(4):
                x = srcs[j]
                for w, fv, dup2 in ((w1s if j == 0 else w1v, f1v, False),
                                    (w2s if j == 0 else w2v, f2v, dup)):
                    for ci, (c, sl) in enumerate(chunks):
                        p = pstile(f"acc{k % 4}{(k // 4) % 2}")
                        k += 1
                        inst = nc.tensor.matmul(p[:U], w[:Min, :U], x[:, sl],
                                                start=True, stop=True)
                        if LDW_SKIP and ci > 0:
                            inst.ins.ldweights = False
                        nc.scalar.copy(fv[:U, j, sl], p[:U])
                        if dup2:
                            nc.scalar.copy(fv[64:128, j, sl], p[:64])
            # stage f1 rows to DRAM [U, 4*BC]
            nc.sync.dma_start(f1d[blk][:U, :], f1_all[:U, :])
            return f1_all, f2_all

        def _network():
            # ---- block 1 factors (plain l1/l2 on input) ----
            f1a, f2a = factors(
                (W["l1_s"], W["l1_v"], W["l2_s"], W["l2_v"]),
                sT, vT, 64, 64, "b1")

            def w1tile(kt):
                wv = wb1[:].rearrange("p (k t w) -> p k t w", k=32, t=4)
                return tuple(wv[:, kt, t, :] for t in range(4))

            tp_s, tp_v, tp_vv = tp_block("b1", 64, 32, f1a, f2a, w1tile)
            tanh_s, gated, gatedv = gate(W["g1_ws"], W["g1_wg"], W["g1_wv"],
                                         tp_s, tp_vv)

            # ---- block 2 factors (fused b1-out + b2-l1/l2) ----
            gsl = [gated[:].rearrange("p (s f) -> p s f", s=3)[:, i, :]
                   for i in range(3)]
            f1b, f2b = factors(
                (W["f2_l1_s"], W["f2_l1_v"], W["f2_l2_s"], W["f2_l2_v"]),
                tanh_s, gsl, 128, 128, "b2")

            wgrp = {}

            def w2tile(kt):
                g = kt // G_W
                if g not in wgrp:
                    wt = wstream.tile([128, G_W * 4 * 128], f16, tag="w2g",
                                      name="w2g")
                    nc.scalar.dma_start(
                        wt[:], wtp2_d[:, g * G_W * 512:(g + 1) * G_W * 512])
                    wgrp[g] = wt
                wv = wgrp[g][:].rearrange("p (k t w) -> p k t w", k=G_W, t=4)
                return tuple(wv[:, kt % G_W, t, :] for t in range(4))

            tp_s2, tp_v2, tp_vv2 = tp_block("b2", 128, 128, f1b, f2b, w2tile)
            tanh_s2, gated2, gatedv2 = gate(W["g2_ws"], W["g2_wg"], W["g2_wv"],
                                            tp_s2, tp_vv2)

            # ---- final fused linears -> out ----
            fo_a = acts.tile([128, BC], f32, tag="fo_a", name="fo_a")
            fo_b = acts.tile([128, BC], f32, tag="fo_b", name="fo_b")
            g2 = gated2[:].rearrange("p (s f) -> p s f", s=3)
            outs = [(W["ff_s"], 0, fo_a, 0), (W["ff_v"], 1, fo_a, 64),
                    (W["ff_v"], 2, fo_b, 0), (W["ff_v"], 3, fo_b, 64)]
            k = 0
            for idx, (w, _, dst, r0) in enumerate(outs):
                for c in range(NCH):
                    sl = slice(c * CH, (c + 1) * CH)
                    x = tanh_s2[:, sl] if idx == 0 else g2[:, idx - 1, sl]
                    p = pstile(f"acc{k % 4}{(k // 4) % 2}")
                    inst = nc.tensor.matmul(p[:64], w[:, :64], x,
                                            start=True, stop=True)
                    if LDW_SKIP and (idx, c) not in ((0, 0), (1, 0)):
                        inst.ins.ldweights = False
                    k += 1
                    nc.scalar.copy(dst[r0:r0 + 64, sl], p[:64])
            nc.sync.dma_start(out_d[0:128, :], fo_a[:])
            nc.sync.dma_start(out_d[128:256, :], fo_b[:])

        if repeat > 1:
            with tc.For_i(0, repeat, 1):
                _network()
        else:
            _network()

    nc.finalize()
    return nc


def _host_prep(inputs):
    """Fold norm constants, fuse out-linears, reorder/cast TP weights."""
    hf = np.float16
    d = {}
    c64 = np.float32(1.0 / np.sqrt(64.0))
    c128 = np.float32(1.0 / np.sqrt(128.0))
    c_og = np.float32(TANH_GAIN / np.sqrt(128.0))

    def tp_cat(blk, M):
        c_tp = 1.0 / (M * np.sqrt(2.0))
        mats = []
        for nm, c in (("ss", c_tp), ("vv", c_tp / np.sqrt(3.0)),
                      ("sv", c_tp), ("vs", c_tp)):
            mats.append((inputs[f"{blk}_tp_{nm}"] * np.float32(c)))
        arr = np.stack(mats, axis=0)          # [path, u, v, w]
        arr = arr.transpose(2, 1, 0, 3)       # [v, u, path, w]
        if M == 128:
            return np.ascontiguousarray(arr.reshape(128, -1)).astype(hf)
        # b1: partition p = r*64+v handles u=2kt+r
        a = arr.reshape(64, 32, 2, 4, 128)     # [v, kt, r, path, w]
        a = a.transpose(2, 0, 1, 3, 4)         # [r, v, kt, path, w]
        return np.ascontiguousarray(a.reshape(128, -1)).astype(hf)

    d["wtp1"] = tp_cat("b1", 64)
    d["wtp2"] = tp_cat("b2", 128)

    d["l1_s"] = (inputs["b1_l1_w0"] * c64).astype(hf)
    d["l1_v"] = (inputs["b1_l1_w1"] * c64).astype(hf)
    d["l2_s"] = (inputs["b1_l2_w0"] * c64).astype(hf)
    d["l2_v"] = (inputs["b1_l2_w1"] * c64).astype(hf)

    for b in ("1", "2"):
        for nm in ("ws", "wg", "wv"):
            d[f"g{b}_{nm}"] = (inputs[f"b{b}_g_{nm}"] * c128).astype(hf)

    # fused block1-out @ block2-l1/l2 (both with their norm constants)
    o0 = inputs["b1_o_w0"] * c_og
    o1 = inputs["b1_o_w1"] * c_og
    d["f2_l1_s"] = (o0 @ (inputs["b2_l1_w0"] * c128)).astype(hf)
    d["f2_l1_v"] = (o1 @ (inputs["b2_l1_w1"] * c128)).astype(hf)
    d["f2_l2_s"] = (o0 @ (inputs["b2_l2_w0"] * c128)).astype(hf)
    d["f2_l2_v"] = (o1 @ (inputs["b2_l2_w1"] * c128)).astype(hf)
    # fused block2-out @ final
    o0 = inputs["b2_o_w0"] * c_og
    o1 = inputs["b2_o_w1"] * c_og
    d["ff_s"] = (o0 @ (inputs["f_w0"] * c128)).astype(hf)
    d["ff_v"] = (o1 @ (inputs["f_w1"] * c128)).astype(hf)
    return d


def _shard_inputs(x, w):
    sT_full = np.ascontiguousarray(x[:, :64].T).astype(np.float16)
    v_full = x[:, 64:].reshape(B, 64, 3)
    vT_full = np.ascontiguousarray(
        v_full.transpose(2, 1, 0)).astype(np.float16)            # [3, 64, B]
    in_maps = []
    for c in range(N_CORES):
        bs = slice(c * BC, (c + 1) * BC)
        m = dict(w)
        m["s0"] = np.ascontiguousarray(sT_full[:, bs])
        m["v0"] = np.ascontiguousarray(vT_full[:, :, bs]).reshape(192, BC)
        in_maps.append(m)
    return in_maps


def kernel(**inputs):
    from concourse.bass_utils import run_bass_kernel_spmd

    x = np.asarray(inputs["x"], dtype=np.float32)
    w = _host_prep({k: np.asarray(v, dtype=np.float32)
                    for k, v in inputs.items() if k != "x"})

    if "nc" not in _CACHE:
        _CACHE["nc"] = _build_program()
    nc = _CACHE["nc"]

    in_maps = _shard_inputs(x, w)
    res = run_bass_kernel_spmd(nc, in_maps, core_ids=list(range(N_CORES)))

    out = np.empty((B, 256), dtype=np.float32)
    for c in range(N_CORES):
        o = res.results[c]["out"]                                # [256, BC]
        bs = slice(c * BC, (c + 1) * BC)
        out[bs, :64] = o[:64].T
        v = o[64:].reshape(3, 64, BC)
        out[bs, 64:] = v.transpose(2, 1, 0).reshape(BC, 192)
    return out
